# revision 7
# baseline (speedup 1.0000x reference)
"""Hopfield neuron update kernel for 8 Trainium2 NeuronCores.

Computes, for W [N,N], s [N] (+-1), b [N]:
    act       = W @ s - diag(W)*s + (N-1)*b
    new_state = where(act >= 0, 1, -1)

Sharding: row-shard W across 8 cores (each core owns N/8=2048 rows of W,
bias and output), replicate s. Per core the matvec runs on the Vector
engine with fused multiply+reduce (scalar_tensor_tensor + accum_out) over
natural-layout W tiles [128 rows x 4096 cols] streamed from HBM on the
sync HWDGE ring at fabric rate (~421 GB/s observed). The replicated state
vector is loaded once as [1, N] (64 KiB) on the scalar HWDGE ring and
broadcast across the 128 SBUF partitions by GpSimd partition_broadcast,
keeping the 8.4 MB of partition-replication off the saturated DMA stream.
The diag/bias correction is folded host-side into c = (N-1)*b - diag*s so
the epilogue is add + sign, written out as one combined DMA.
"""

import os
import sys

import numpy as np

for _p in ("/opt/trn_rl_repo", "/root/.axon_site/_ro/trn_rl_repo"):
    if os.path.isdir(_p) and _p not in sys.path:
        sys.path.insert(0, _p)

N = 16384
NCORES = 8
R = N // NCORES          # rows per core: 2048
P = 128                  # SBUF partitions
G = R // P               # row groups per core: 16
FD = 4096                # DMA tile free size (16 KiB/partition, contiguous)
NCHUNK = N // FD         # tiles (and accum slots) per row group: 4
WBUFS = 8                # in-flight W tiles (DMA prefetch depth, 16 MiB)
NBC = 4                  # partition_broadcast chunks for s

_CACHE = {}


def _build_nc():
    import concourse.bacc as bacc
    import concourse.mybir as mybir
    from concourse.tile import TileContext

    f32 = mybir.dt.float32
    nc = bacc.Bacc()

    w = nc.dram_tensor("w", [R, N], f32, kind="ExternalInput")
    s = nc.dram_tensor("s", [N], f32, kind="ExternalInput")
    c_t = nc.dram_tensor("c_t", [P, G], f32, kind="ExternalInput")
    out_o = nc.dram_tensor("out_o", [P, 2, G], f32, kind="ExternalOutput")

    with TileContext(nc) as tc:
        with (
            tc.tile_pool(name="consts", bufs=1) as consts,
            tc.tile_pool(name="wpool", bufs=WBUFS) as wpool,
        ):
            sb = consts.tile([P, N], f32)
            partials = consts.tile([P, G, NCHUNK], f32)
            dummy = consts.tile([P, 1], f32)
            ones = consts.tile([1, P], f32)

            # s -> partition 0 of sb (64 KiB, scalar HWDGE ring so it does
            # not queue ahead of W tiles on the sync ring), then broadcast
            # to all 128 partitions without touching the saturated DMA
            # fabric: rank-1 outer product ones[128,1] @ s[1,512] on the
            # idle TensorE into PSUM, copied to SBUF by the idle ACT
            # engine, chunked so compute can start early. The copy also
            # rewrites partition 0 with the identical values (WAR dep on
            # the matmul read keeps it safe).
            nc.vector.memset(ones[:], 1.0)
            nc.scalar.dma_start(out=sb[0:1, :], in_=s[None, :])
            BCF = 512
            with tc.tile_pool(name="bcpsum", bufs=4, space="PSUM") as bcpsum:
                for k in range(N // BCF):
                    js = slice(k * BCF, (k + 1) * BCF)
                    pt = bcpsum.tile([P, BCF], f32)
                    nc.tensor.matmul(pt[:], ones[:], sb[0:1, js])
                    nc.scalar.copy(out=sb[:, js], in_=pt[:])

            # Stream W and accumulate dot products per 128-row group.
            for g in range(G):
                rows = slice(g * P, (g + 1) * P)
                for cd in range(NCHUNK):
                    js = slice(cd * FD, (cd + 1) * FD)
                    wt = wpool.tile([P, FD], f32)
                    nc.sync.dma_start(out=wt[:], in_=w[rows, js])
                    nc.vector.scalar_tensor_tensor(
                        out=dummy[:].broadcast_to([P, FD]),
                        in0=wt[:],
                        scalar=1.0,
                        in1=sb[:, js],
                        op0=mybir.AluOpType.bypass,
                        op1=mybir.AluOpType.mult,
                        accum_out=partials[:, g, cd : cd + 1],
                    )

            # Epilogue: act = sum(partials) + c; ns = 2*(act>=0) - 1.
            ct = consts.tile([P, G], f32)
            out_sb = consts.tile([P, 2, G], f32)
            acc = consts.tile([P, G, 1], f32)
            ns0 = consts.tile([P, G], f32)
            nc.scalar.dma_start(out=ct[:], in_=c_t[:, :])
            nc.vector.tensor_reduce(
                out=acc[:],
                in_=partials[:],
                axis=mybir.AxisListType.X,
                op=mybir.AluOpType.add,
            )
            nc.vector.tensor_tensor(
                out=out_sb[:, 0, :],
                in0=acc[:, :, 0],
                in1=ct[:],
                op=mybir.AluOpType.add,
            )
            nc.vector.tensor_scalar(
                out=ns0[:],
                in0=out_sb[:, 0, :],
                scalar1=0.0,
                scalar2=None,
                op0=mybir.AluOpType.is_ge,
            )
            nc.vector.tensor_scalar(
                out=out_sb[:, 1, :],
                in0=ns0[:],
                scalar1=2.0,
                scalar2=-1.0,
                op0=mybir.AluOpType.mult,
                op1=mybir.AluOpType.add,
            )
            nc.scalar.dma_start(out=out_o[:, :, :], in_=out_sb[:])

    nc.finalize()
    return nc


def get_nc():
    if "nc" not in _CACHE:
        _CACHE["nc"] = _build_nc()
    return _CACHE["nc"]


def make_in_maps(weights, state, bias):
    weights = np.ascontiguousarray(weights, dtype=np.float32)
    state = np.ascontiguousarray(state, dtype=np.float32)
    bias = np.ascontiguousarray(bias, dtype=np.float32)
    diag = np.ascontiguousarray(np.diagonal(weights))
    corr = (N - 1) * bias - diag * state
    in_maps = []
    for c in range(NCORES):
        rows = slice(c * R, (c + 1) * R)
        in_maps.append(
            {
                "w": weights[rows],
                "s": state,
                "c_t": np.ascontiguousarray(corr[rows].reshape(G, P).T),
            }
        )
    return in_maps


def gather(results):
    act = np.concatenate(
        [r["out_o"][:, 0, :].T.reshape(R) for r in results]
    )
    ns = np.concatenate(
        [r["out_o"][:, 1, :].T.reshape(R) for r in results]
    )
    return act.astype(np.float32), ns.astype(np.float32)


def kernel(weights, state, bias):
    from concourse.bass_utils import run_bass_kernel_spmd

    nc = get_nc()
    in_maps = make_in_maps(weights, state, bias)
    res = run_bass_kernel_spmd(nc, in_maps, list(range(NCORES)))
    return gather(res.results)
